# revision 3
# baseline (speedup 1.0000x reference)
"""LIF spiking-neuron recurrence kernel for Trainium2 (Bass/Tile, 8-core SPMD).

Problem: x [32, 128, 32, 32, 8] f32, time on the LAST axis (T=8).
    u_0 = x_0;  o_t = (u_t > Vth);  u_{t+1} = TAU * u_t * (1 - o_t) + x_{t+1}
Output: spikes o [32, 128, 32, 32, 8] f32 (0.0 / 1.0).

Sharding: pure data-parallel over the batch dim (32 -> 4 per core, 8 cores),
no communication. The host lays each core's shard out t-plane-major
([pixels, T] -> [T, pixels] per 1024-pixel row group) so every on-chip
operand is dense unit-stride.

The kernel is HBM-bandwidth-bound (per-core peak ~358 GB/s). Spikes are
exactly 0/1, so the output is stored as int8 sign values (-1/0/+1), cutting
store traffic 4x vs f32: per-core traffic drops from 33.5 MB to 21 MB.
The host maps sign>0 -> 1.0f on the way out.

To keep the engines under the new ~59us DMA floor, per-timestep work is
spread across three engines (all costs for a [128, 2048] f32 plane):
    ACT:    o_t  = sign(u - Vth)              -> int8, also issues the store
    DVE:    m2   = (u <= Vth) * TAU           tensor_scalar, 2x mode, ~1.1us
    DVE:    w    = u * m2                     tensor_tensor 1x, ~2.3us
    DVE+GPSIMD:  u' = w + x_{t+1}             split by columns (D on DVE)
Multiplying by m2 in {0.0, TAU} rounds identically to the reference's
TAU*u*(1-o), so results stay bit-exact.

Timesteps are emitted t-outer / tile-inner so each engine's in-order queue
interleaves the two row tiles: DVE runs tile B's mask/mult while GPSIMD adds
tile A, hiding the cross-engine dependency chain.
"""

import numpy as np

import bass_rust
import concourse.bass as bass
import concourse.mybir as mybir
import concourse.tile as tile
from concourse.bass_utils import run_bass_kernel_spmd

VTH = 0.2
TAU = 0.25

N_CORES = 8
FULL_SHAPE = (32, 128, 32, 32, 8)
B_PER_CORE = FULL_SHAPE[0] // N_CORES  # 4
T = FULL_SHAPE[-1]  # 8

ROWS = 256  # per-core partition rows: 4*128*32*32*8 / FREE
FREE = 16384  # free dim per row
C = FREE // T  # 2048 pixels per partition row
N_TILES = ROWS // 128  # 2

# Columns of each state-update add computed on DVE; the rest go to GPSIMD.
# GPSIMD's 2-input elementwise floor is ~2.5 cyc/elem @1.2GHz vs DVE's
# 1 cyc/elem @0.96GHz, so a small DVE share balances the two queues.
D_ADD = 256

_cache: dict = {}


def _split_multi_waits(nc: bass.Bass) -> int:
    """Hoist all-but-one embedded sync waits onto standalone EventSemaphore
    instructions. The walrus build behind bass2jax rejects >1 sync wait per
    instruction ("Too many sync wait commands"); a standalone wait on the
    same engine stream immediately before is semantically identical."""
    n = 0
    for fn in nc.m.functions:
        for block in fn.blocks:
            out = []
            changed = False
            for ins in block.instructions:
                si = ins.sync_info
                waits = list(si.on_wait) if si is not None else []
                if len(waits) > 1:
                    for k, w in enumerate(waits[:-1]):
                        ev = mybir.InstEventSemaphore(
                            name=f"{ins.name}-hw{k}", ins=[], outs=[]
                        )
                        ev.sync_info = bass_rust.SyncInfo(
                            on_wait=[w], on_update=[]
                        )
                        ev.engine = ins.engine
                        nc.inst_map[ev.name] = ev
                        out.append(ev)
                        n += 1
                    si.on_wait = [waits[-1]]
                    changed = True
                out.append(ins)
            if changed:
                block.instructions = out
    return n


def _build_bass() -> bass.Bass:
    f32 = mybir.dt.float32
    i8 = mybir.dt.int8
    Alu = mybir.AluOpType
    Act = mybir.ActivationFunctionType

    nc = bass.Bass(trn_type="TRN2")
    x_d = nc.dram_tensor("x", [ROWS, FREE], f32, kind="ExternalInput")
    y_d = nc.dram_tensor("y", [ROWS, FREE], i8, kind="ExternalOutput")

    # The Sign activation takes its bias as a per-partition const AP; only
    # 0.0/1.0 are pre-registered, so register -VTH the same way Bass does.
    bias_t = nc.alloc_sbuf_tensor("const-neg-vth", [128, 1], f32)
    nc.gpsimd.memset(bias_t.ap(), -VTH)
    nc.const_aps.aps[(f32, -VTH)] = bias_t.ap()
    nc.all_engine_barrier()

    with tile.TileContext(nc) as tc:
        with (
            tc.tile_pool(name="pin", bufs=12) as pin,
            tc.tile_pool(name="pout", bufs=4) as pout,
            tc.tile_pool(name="pm", bufs=3) as pm,
            tc.tile_pool(name="pw", bufs=3) as pw,
            tc.tile_pool(name="pu", bufs=4) as pu,
        ):
            row_sl = [slice(i * 128, (i + 1) * 128) for i in range(N_TILES)]

            # All plane loads issued up front, in consumption order
            # (t-outer, tile-inner) so the 12-deep buffer ring frees in
            # FIFO order while later loads overlap compute.
            xp = [[None] * T for _ in range(N_TILES)]
            for t in range(T):
                for i in range(N_TILES):
                    p = pin.tile([128, C], f32, tag="xp")
                    nc.sync.dma_start(p, x_d[row_sl[i], t * C : (t + 1) * C])
                    xp[i][t] = p

            u = [xp[i][0] for i in range(N_TILES)]
            for t in range(T):
                for i in range(N_TILES):
                    cols = slice(t * C, (t + 1) * C)
                    # ACT: spike output as sign(u - Vth) in {-1, 0, +1},
                    # cast to int8; host maps >0 to 1.0. ACT also issues
                    # the store so SP's issue queue only carries loads.
                    o_t = pout.tile([128, C], i8, tag="o")
                    nc.scalar.activation(
                        o_t, u[i], Act.Sign, bias=-VTH, scale=1.0
                    )
                    nc.scalar.dma_start(y_d[row_sl[i], cols], o_t)

                    if t == T - 1:
                        continue
                    # DVE: m2 = (u <= Vth) * TAU in {0.0, TAU}
                    m2 = pm.tile([128, C], f32, tag="m")
                    nc.vector.tensor_scalar(
                        m2, u[i], VTH, TAU, Alu.is_le, Alu.mult
                    )
                    # DVE: w = u * m2  (== TAU*u*(1-o), bit-exact)
                    w = pw.tile([128, C], f32, tag="w")
                    nc.vector.tensor_tensor(w, u[i], m2, Alu.mult)
                    # u' = w + x_{t+1}: columns [0,D) on DVE, rest on GPSIMD
                    un = pu.tile([128, C], f32, tag="u")
                    xn = xp[i][t + 1]
                    nc.vector.tensor_tensor(
                        un[:, :D_ADD], w[:, :D_ADD], xn[:, :D_ADD], Alu.add
                    )
                    nc.gpsimd.tensor_tensor(
                        un[:, D_ADD:], w[:, D_ADD:], xn[:, D_ADD:], Alu.add
                    )
                    u[i] = un

    _split_multi_waits(nc)
    return nc


def _shard(x: np.ndarray, c: int) -> np.ndarray:
    """Core c's shard, t-plane-major: [ROWS, C, T] -> [ROWS, T, C] -> flat."""
    s = x[c * B_PER_CORE : (c + 1) * B_PER_CORE].reshape(ROWS, C, T)
    return np.ascontiguousarray(s.transpose(0, 2, 1)).reshape(ROWS, FREE)


def _unshard(y: np.ndarray) -> np.ndarray:
    """Invert _shard's layout for one core's int8 sign output -> f32 0/1."""
    o = (y > 0).astype(np.float32)
    s = o.reshape(ROWS, T, C).transpose(0, 2, 1)
    return np.ascontiguousarray(s).reshape(B_PER_CORE, *FULL_SHAPE[1:])


def kernel(x: np.ndarray) -> np.ndarray:
    assert x.shape == FULL_SHAPE, x.shape
    in_dtype = x.dtype

    if "nc" not in _cache:
        _cache["nc"] = _build_bass()
    nc = _cache["nc"]

    x = np.ascontiguousarray(x, dtype=np.float32)
    in_maps = [{"x": _shard(x, c)} for c in range(N_CORES)]
    res = run_bass_kernel_spmd(nc, in_maps, core_ids=list(range(N_CORES)))
    out = np.concatenate(
        [_unshard(res.results[c]["y"]) for c in range(N_CORES)], axis=0
    )
    return out.astype(in_dtype, copy=False)


# revision 5
# speedup vs baseline: 1.0918x; 1.0918x over previous
"""LIF spiking-neuron recurrence kernel for Trainium2 (Bass/Tile, 8-core SPMD).

Problem: x [32, 128, 32, 32, 8] f32, time on the LAST axis (T=8).
    u_0 = x_0;  o_t = (u_t > Vth);  u_{t+1} = TAU * u_t * (1 - o_t) + x_{t+1}
Output: spikes o [32, 128, 32, 32, 8] f32 (0.0 / 1.0).

Sharding: pure data-parallel over the batch dim (32 -> 4 per core, 8 cores),
no communication. The host lays each core's shard out t-plane-major
([pixels, T] -> [T, pixels] per 1024-pixel row group) so every on-chip
operand is dense unit-stride.

The kernel is HBM-bandwidth-bound (per-core peak ~358 GB/s). Spikes are
exactly 0/1, so the output is stored as int8 sign values (-1/0/+1), cutting
store traffic 4x vs f32: per-core traffic drops from 33.5 MB to 21 MB.
The host maps sign>0 -> 1.0f on the way out.

To keep the engines under the new ~59us DMA floor, per-timestep work is
spread across three engines (all costs for a [128, 2048] f32 plane):
    ACT:    o_t  = sign(u - Vth)              -> int8, also issues the store
    DVE:    m2   = (u <= Vth) * TAU           tensor_scalar, 2x mode, ~1.1us
    DVE:    w    = u * m2                     tensor_tensor 1x, ~2.3us
    DVE+GPSIMD:  u' = w + x_{t+1}             split by columns (D on DVE)
Multiplying by m2 in {0.0, TAU} rounds identically to the reference's
TAU*u*(1-o), so results stay bit-exact.

Timesteps are emitted t-outer / tile-inner so each engine's in-order queue
interleaves the two row tiles: DVE runs tile B's mask/mult while GPSIMD adds
tile A, hiding the cross-engine dependency chain.
"""

import numpy as np

import bass_rust
import concourse.bass as bass
import concourse.mybir as mybir
import concourse.tile as tile
from concourse.bass_utils import run_bass_kernel_spmd

VTH = 0.2
TAU = 0.25

N_CORES = 8
FULL_SHAPE = (32, 128, 32, 32, 8)
B_PER_CORE = FULL_SHAPE[0] // N_CORES  # 4
T = FULL_SHAPE[-1]  # 8

ROWS = 256  # per-core partition rows: 4*128*32*32*8 / FREE
FREE = 16384  # free dim per row
C = FREE // T  # 2048 pixels per partition row
N_TILES = ROWS // 128  # 2

# Columns of each state-update add computed on DVE; the rest go to GPSIMD.
# Measured: GPSIMD's 2-input elementwise runs ~3.4 cyc/elem @1.2GHz and its
# SBUF port is shared with DVE (2-port DVE ops degrade while GPSIMD runs),
# so only a small slice (if any) should go there. 2048 = all on DVE.
D_ADD = 2048

_cache: dict = {}


def _split_multi_waits(nc: bass.Bass) -> int:
    """Hoist all-but-one embedded sync waits onto standalone EventSemaphore
    instructions. The walrus build behind bass2jax rejects >1 sync wait per
    instruction ("Too many sync wait commands"); a standalone wait on the
    same engine stream immediately before is semantically identical."""
    n = 0
    for fn in nc.m.functions:
        for block in fn.blocks:
            out = []
            changed = False
            for ins in block.instructions:
                si = ins.sync_info
                waits = list(si.on_wait) if si is not None else []
                if len(waits) > 1:
                    for k, w in enumerate(waits[:-1]):
                        ev = mybir.InstEventSemaphore(
                            name=f"{ins.name}-hw{k}", ins=[], outs=[]
                        )
                        ev.sync_info = bass_rust.SyncInfo(
                            on_wait=[w], on_update=[]
                        )
                        ev.engine = ins.engine
                        nc.inst_map[ev.name] = ev
                        out.append(ev)
                        n += 1
                    si.on_wait = [waits[-1]]
                    changed = True
                out.append(ins)
            if changed:
                block.instructions = out
    return n


def _build_bass() -> bass.Bass:
    f32 = mybir.dt.float32
    i8 = mybir.dt.int8
    Alu = mybir.AluOpType
    Act = mybir.ActivationFunctionType

    nc = bass.Bass(trn_type="TRN2")
    x_d = nc.dram_tensor("x", [ROWS, FREE], f32, kind="ExternalInput")
    y_d = nc.dram_tensor("y", [ROWS, FREE], i8, kind="ExternalOutput")

    # The Sign activation takes its bias as a per-partition const AP; only
    # 0.0/1.0 are pre-registered, so register -VTH the same way Bass does.
    bias_t = nc.alloc_sbuf_tensor("const-neg-vth", [128, 1], f32)
    nc.gpsimd.memset(bias_t.ap(), -VTH)
    nc.const_aps.aps[(f32, -VTH)] = bias_t.ap()
    nc.all_engine_barrier()

    with tile.TileContext(nc) as tc:
        with (
            tc.tile_pool(name="pin", bufs=12) as pin,
            tc.tile_pool(name="pout", bufs=4) as pout,
            tc.tile_pool(name="pm", bufs=3) as pm,
            tc.tile_pool(name="pw", bufs=3) as pw,
            tc.tile_pool(name="pu", bufs=4) as pu,
        ):
            row_sl = [slice(i * 128, (i + 1) * 128) for i in range(N_TILES)]

            # All plane loads issued up front, in consumption order
            # (t-outer, tile-inner) so the 12-deep buffer ring frees in
            # FIFO order while later loads overlap compute.
            xp = [[None] * T for _ in range(N_TILES)]
            for t in range(T):
                for i in range(N_TILES):
                    p = pin.tile([128, C], f32, tag="xp")
                    nc.sync.dma_start(p, x_d[row_sl[i], t * C : (t + 1) * C])
                    xp[i][t] = p

            u = [xp[i][0] for i in range(N_TILES)]
            for t in range(T):
                for i in range(N_TILES):
                    cols = slice(t * C, (t + 1) * C)
                    # ACT: spike output as sign(u - Vth) in {-1, 0, +1},
                    # cast to int8; host maps >0 to 1.0. ACT also issues
                    # the store so SP's issue queue only carries loads.
                    o_t = pout.tile([128, C], i8, tag="o")
                    nc.scalar.activation(
                        o_t, u[i], Act.Sign, bias=-VTH, scale=1.0
                    )
                    nc.scalar.dma_start(y_d[row_sl[i], cols], o_t)

                    if t == T - 1:
                        continue
                    # DVE: m2 = (u <= Vth) * TAU in {0.0, TAU}
                    m2 = pm.tile([128, C], f32, tag="m")
                    nc.vector.tensor_scalar(
                        m2, u[i], VTH, TAU, Alu.is_le, Alu.mult
                    )
                    # DVE: w = u * m2  (== TAU*u*(1-o), bit-exact)
                    w = pw.tile([128, C], f32, tag="w")
                    nc.vector.tensor_tensor(w, u[i], m2, Alu.mult)
                    # u' = w + x_{t+1}: columns [0,D) on DVE, rest on GPSIMD
                    un = pu.tile([128, C], f32, tag="u")
                    xn = xp[i][t + 1]
                    nc.vector.tensor_tensor(
                        un[:, :D_ADD], w[:, :D_ADD], xn[:, :D_ADD], Alu.add
                    )
                    if D_ADD < C:
                        nc.gpsimd.tensor_tensor(
                            un[:, D_ADD:], w[:, D_ADD:], xn[:, D_ADD:], Alu.add
                        )
                    u[i] = un

    _split_multi_waits(nc)
    return nc


def _shard(x: np.ndarray, c: int) -> np.ndarray:
    """Core c's shard, t-plane-major: [ROWS, C, T] -> [ROWS, T, C] -> flat."""
    s = x[c * B_PER_CORE : (c + 1) * B_PER_CORE].reshape(ROWS, C, T)
    return np.ascontiguousarray(s.transpose(0, 2, 1)).reshape(ROWS, FREE)


def _unshard(y: np.ndarray) -> np.ndarray:
    """Invert _shard's layout for one core's int8 sign output -> f32 0/1."""
    o = (y > 0).astype(np.float32)
    s = o.reshape(ROWS, T, C).transpose(0, 2, 1)
    return np.ascontiguousarray(s).reshape(B_PER_CORE, *FULL_SHAPE[1:])


def kernel(x: np.ndarray) -> np.ndarray:
    assert x.shape == FULL_SHAPE, x.shape
    in_dtype = x.dtype

    if "nc" not in _cache:
        _cache["nc"] = _build_bass()
    nc = _cache["nc"]

    x = np.ascontiguousarray(x, dtype=np.float32)
    in_maps = [{"x": _shard(x, c)} for c in range(N_CORES)]
    res = run_bass_kernel_spmd(nc, in_maps, core_ids=list(range(N_CORES)))
    out = np.concatenate(
        [_unshard(res.results[c]["y"]) for c in range(N_CORES)], axis=0
    )
    return out.astype(in_dtype, copy=False)


# revision 6
# speedup vs baseline: 1.2364x; 1.1324x over previous
"""LIF spiking-neuron recurrence kernel for Trainium2 (Bass/Tile, 8-core SPMD).

Problem: x [32, 128, 32, 32, 8] f32, time on the LAST axis (T=8).
    u_0 = x_0;  o_t = (u_t > Vth);  u_{t+1} = TAU * u_t * (1 - o_t) + x_{t+1}
Output: spikes o [32, 128, 32, 32, 8] f32 (0.0 / 1.0).

Sharding: pure data-parallel over the batch dim (32 -> 4 per core, 8 cores),
no communication. While sharding, the host also lays each core's shard out
t-plane-major ([pixels, T] -> [T, pixels] per 1024-pixel row group) so every
on-chip operand is dense unit-stride.

The kernel is HBM-bandwidth-bound (per-core peak ~358 GB/s). Spikes are
exactly 0/1, so the output is stored as int8 (0/1), cutting store traffic
4x vs f32: per-core HBM traffic drops from 33.5 MB (~94 us floor) to
21 MB (~59 us floor). The host casts back to f32 on the way out.

Per-timestep compute (on [128, C] dense views):
    m   = (u <= Vth)                  DVE tensor_scalar (is_le), 2x f32 mode
    o_t = 1 - m  -> int8              ACT activation(Copy, scale=-1, bias=1)
    w   = (u * TAU) * m               DVE scalar_tensor_tensor (mult, mult)
    u   = w + x_{t+1}                 DVE tensor_tensor add

Multiplying by m in {0.0, 1.0} is exact, so results are bit-identical to the
reference ordering TAU*u*(1-o) + x.
"""

import numpy as np

import bass_rust
import concourse.bass as bass
import concourse.mybir as mybir
import concourse.tile as tile
from concourse.bass_utils import run_bass_kernel_spmd

VTH = 0.2
TAU = 0.25

N_CORES = 8
FULL_SHAPE = (32, 128, 32, 32, 8)
B_PER_CORE = FULL_SHAPE[0] // N_CORES  # 4
T = FULL_SHAPE[-1]  # 8

ROWS = 256  # per-core partition rows: 4*128*32*32*8 / FREE
FREE = 16384  # free dim per row
C = FREE // T  # 2048 pixels per partition row
N_TILES = ROWS // 128  # 2

_cache: dict = {}


def _split_multi_waits(nc: bass.Bass) -> int:
    """Hoist all-but-one embedded sync waits onto standalone EventSemaphore
    instructions. The walrus build behind bass2jax rejects >1 sync wait per
    instruction ("Too many sync wait commands"); a standalone wait on the
    same engine stream immediately before is semantically identical."""
    n = 0
    for fn in nc.m.functions:
        for block in fn.blocks:
            out = []
            changed = False
            for ins in block.instructions:
                si = ins.sync_info
                waits = list(si.on_wait) if si is not None else []
                if len(waits) > 1:
                    for k, w in enumerate(waits[:-1]):
                        ev = mybir.InstEventSemaphore(
                            name=f"{ins.name}-hw{k}", ins=[], outs=[]
                        )
                        ev.sync_info = bass_rust.SyncInfo(
                            on_wait=[w], on_update=[]
                        )
                        ev.engine = ins.engine
                        nc.inst_map[ev.name] = ev
                        out.append(ev)
                        n += 1
                    si.on_wait = [waits[-1]]
                    changed = True
                out.append(ins)
            if changed:
                block.instructions = out
    return n


def _build_bass() -> bass.Bass:
    f32 = mybir.dt.float32
    i8 = mybir.dt.int8
    Alu = mybir.AluOpType
    Act = mybir.ActivationFunctionType

    nc = bass.Bass(trn_type="TRN2")
    x_d = nc.dram_tensor("x", [ROWS, FREE], f32, kind="ExternalInput")
    y_d = nc.dram_tensor("y", [ROWS, FREE], i8, kind="ExternalOutput")

    with tile.TileContext(nc) as tc:
        with (
            tc.tile_pool(name="pin", bufs=12) as pin,
            tc.tile_pool(name="pout", bufs=4) as pout,
            tc.tile_pool(name="pm", bufs=3) as pm,
            tc.tile_pool(name="ptmp", bufs=2) as ptmp,
        ):
            for i in range(N_TILES):
                rows = slice(i * 128, (i + 1) * 128)
                # per-t-plane loads: compute starts after plane 0 lands,
                # instead of stalling on one monolithic 4 MiB transfer
                xp = []
                for t in range(T):
                    p = pin.tile([128, C], f32, tag="xp")
                    nc.sync.dma_start(p, x_d[rows, t * C : (t + 1) * C])
                    xp.append(p)

                u = ptmp.tile([128, C], f32, tag="u")
                w = ptmp.tile([128, C], f32, tag="w")
                for t in range(T - 1):
                    u_src = xp[0] if t == 0 else u
                    m = pm.tile([128, C], f32, tag="m")
                    o_t = pout.tile([128, C], i8, tag="op")
                    # m = (u <= Vth) in {0.0, 1.0}
                    nc.vector.tensor_scalar(m, u_src, VTH, None, Alu.is_le)
                    # o_t = 1 - m, cast to int8 {0, 1}
                    nc.scalar.activation(o_t, m, Act.Copy, bias=1.0, scale=-1.0)
                    # per-plane store drains while later steps still compute;
                    # issued from ACT (also HWDGE) so SP's issue queue — which
                    # serializes at ~0.6us per dma_start — only carries loads
                    nc.scalar.dma_start(y_d[rows, t * C : (t + 1) * C], o_t)
                    # w = (u * TAU) * m
                    nc.vector.scalar_tensor_tensor(
                        w, u_src, TAU, m, Alu.mult, Alu.mult
                    )
                    # u = w + x_{t+1}
                    nc.vector.tensor_tensor(u, w, xp[t + 1], Alu.add)

                # t = T-1: no state update needed, so skip m/ACT and emit
                # o = (u > Vth) straight from DVE in two half-planes whose
                # stores overlap — keeps the kernel tail short
                H = C // 2
                for h in range(2):
                    o_t = pout.tile([128, H], i8, tag="oh")
                    cols = slice(h * H, (h + 1) * H)
                    nc.vector.tensor_scalar(
                        o_t, u[:, cols], VTH, None, Alu.is_gt
                    )
                    nc.sync.dma_start(
                        y_d[rows, (T - 1) * C + h * H : (T - 1) * C + (h + 1) * H],
                        o_t,
                    )

    _split_multi_waits(nc)
    return nc


def _shard(x: np.ndarray, c: int) -> np.ndarray:
    """Core c's shard, t-plane-major: [ROWS, C, T] -> [ROWS, T, C] -> flat."""
    s = x[c * B_PER_CORE : (c + 1) * B_PER_CORE].reshape(ROWS, C, T)
    return np.ascontiguousarray(s.transpose(0, 2, 1)).reshape(ROWS, FREE)


def _unshard(y: np.ndarray) -> np.ndarray:
    """Invert _shard's layout for one core's int8 0/1 output -> f32."""
    o = (y > 0).astype(np.float32)
    s = o.reshape(ROWS, T, C).transpose(0, 2, 1)
    return np.ascontiguousarray(s).reshape(B_PER_CORE, *FULL_SHAPE[1:])


def kernel(x: np.ndarray) -> np.ndarray:
    assert x.shape == FULL_SHAPE, x.shape
    in_dtype = x.dtype

    if "nc" not in _cache:
        _cache["nc"] = _build_bass()
    nc = _cache["nc"]

    x = np.ascontiguousarray(x, dtype=np.float32)
    in_maps = [{"x": _shard(x, c)} for c in range(N_CORES)]
    res = run_bass_kernel_spmd(nc, in_maps, core_ids=list(range(N_CORES)))
    out = np.concatenate(
        [_unshard(res.results[c]["y"]) for c in range(N_CORES)], axis=0
    )
    return out.astype(in_dtype, copy=False)


# revision 7
# speedup vs baseline: 1.4506x; 1.1733x over previous
"""LIF spiking-neuron recurrence kernel for Trainium2 (Bass/Tile, 8-core SPMD).

Problem: x [32, 128, 32, 32, 8] f32, time on the LAST axis (T=8).
    u_0 = x_0;  o_t = (u_t > Vth);  u_{t+1} = TAU * u_t * (1 - o_t) + x_{t+1}
Output: spikes o [32, 128, 32, 32, 8] f32 (0.0 / 1.0).

Sharding: pure data-parallel over the batch dim (32 -> 4 per core, 8 cores),
no communication. The host lays each core's shard out t-plane-major
([pixels, T] -> [T, pixels] per 1024-pixel row group) so every on-chip
operand is dense unit-stride. Spikes are exactly 0/1 so the output is
stored as int8, cutting store traffic 4x (per-core HBM traffic 21 MB,
~59 us floor at the ~358 GB/s per-core peak).

With cheap stores, the binding constraint is the Vector engine: fp32
tensor_tensor runs at 1 elem/cycle/lane, and the recurrence nominally needs
three 2-tensor ops per timestep (mask, masked-mult, add) = ~5.8 us per
[128, 2048] plane. This kernel reduces DVE to TWO ops per step by computing
the spike gate on the otherwise-idle Activation engine as a Relu ramp:

    z   = Relu(-BIG*u + BIG*u*)        ACT; u* = nextafter(Vth) so that
                                       z > 0  <=>  u <= Vth (exact for every
                                       f32 input; z >= ~12 whenever nonzero)
    w   = min(TAU*u, z)                DVE scalar_tensor_tensor (mult, min)
                                       == TAU*u*(u<=Vth) exactly, because
                                       TAU*u <= 0.0500000008 < 12 when gated
    o_t = Relu(1 - z) -> int8          ACT; z==0 -> 1, z>=12 -> 0
    u'  = w + x_{t+1}                  DVE tensor_tensor add

TAU*u rounds identically to the reference's TAU*u*(1-o) path, so spike
outputs are bit-exact (including u == Vth exactly, handled by u*).

Stores issue from the idle GPSIMD HWDGE queue and loads from SP, keeping
both off the two busy compute queues; loads are prefetched two steps ahead
instead of all up front so DMA SBUF-write pressure is spread evenly.
"""

import numpy as np

import bass_rust
import concourse.bass as bass
import concourse.mybir as mybir
import concourse.tile as tile
from concourse.bass_utils import run_bass_kernel_spmd

VTH = 0.2
TAU = 0.25

# Gate constants: BIG*(u* - u) with u* = nextafter(f32(0.2)). fma(-BIG, u, B)
# is > 0 exactly when u <= f32(0.2) and otherwise 0 after Relu; the smallest
# positive value it takes is ~12 (one f32 ulp at 0.2 scaled by BIG), safely
# above max(TAU*u) = 0.0500000008, so min(TAU*u, z) never picks z when gated
# on. Exact under both fused and round-between multiply-add.
BIG = 1.0e9
B_GATE = 200000016.0  # float32(BIG * nextafter(float32(0.2)))

N_CORES = 8
FULL_SHAPE = (32, 128, 32, 32, 8)
B_PER_CORE = FULL_SHAPE[0] // N_CORES  # 4
T = FULL_SHAPE[-1]  # 8

ROWS = 256  # per-core partition rows: 4*128*32*32*8 / FREE
FREE = 16384  # free dim per row
C = FREE // T  # 2048 pixels per partition row
N_TILES = ROWS // 128  # 2

_cache: dict = {}


def _split_multi_waits(nc: bass.Bass) -> int:
    """Hoist all-but-one embedded sync waits onto standalone EventSemaphore
    instructions. The walrus build behind bass2jax rejects >1 sync wait per
    instruction ("Too many sync wait commands"); a standalone wait on the
    same engine stream immediately before is semantically identical."""
    n = 0
    for fn in nc.m.functions:
        for block in fn.blocks:
            out = []
            changed = False
            for ins in block.instructions:
                si = ins.sync_info
                waits = list(si.on_wait) if si is not None else []
                if len(waits) > 1:
                    for k, w in enumerate(waits[:-1]):
                        ev = mybir.InstEventSemaphore(
                            name=f"{ins.name}-hw{k}", ins=[], outs=[]
                        )
                        ev.sync_info = bass_rust.SyncInfo(
                            on_wait=[w], on_update=[]
                        )
                        ev.engine = ins.engine
                        nc.inst_map[ev.name] = ev
                        out.append(ev)
                        n += 1
                    si.on_wait = [waits[-1]]
                    changed = True
                out.append(ins)
            if changed:
                block.instructions = out
    return n


def _build_bass() -> bass.Bass:
    f32 = mybir.dt.float32
    i8 = mybir.dt.int8
    Alu = mybir.AluOpType
    Act = mybir.ActivationFunctionType

    nc = bass.Bass(trn_type="TRN2")
    x_d = nc.dram_tensor("x", [ROWS, FREE], f32, kind="ExternalInput")
    y_d = nc.dram_tensor("y", [ROWS, FREE], i8, kind="ExternalOutput")

    # Non-Copy activations take their bias as a per-partition const AP; only
    # 0.0/1.0 are pre-registered, so add the gate bias the same way Bass does.
    bias_t = nc.alloc_sbuf_tensor("const-bgate", [128, 1], f32)
    nc.gpsimd.memset(bias_t.ap(), B_GATE)
    nc.const_aps.aps[(f32, B_GATE)] = bias_t.ap()
    nc.all_engine_barrier()

    with tile.TileContext(nc) as tc:
        with (
            tc.tile_pool(name="pin", bufs=10) as pin,
            tc.tile_pool(name="pout", bufs=4) as pout,
            tc.tile_pool(name="pz", bufs=4) as pz,
            tc.tile_pool(name="pw", bufs=3) as pw,
            tc.tile_pool(name="pu", bufs=3) as pu,
        ):
            row_sl = [slice(i * 128, (i + 1) * 128) for i in range(N_TILES)]

            def load(i, t):
                p = pin.tile([128, C], f32, tag="xp")
                nc.sync.dma_start(p, x_d[row_sl[i], t * C : (t + 1) * C])
                return p

            # two-step prefetch: planes 0/1 of both tiles land first, the
            # rest stream in one step ahead of their consuming add
            xp = [[None] * T for _ in range(N_TILES)]
            for t in range(2):
                for i in range(N_TILES):
                    xp[i][t] = load(i, t)

            u = [xp[i][0] for i in range(N_TILES)]
            for t in range(T):
                for i in range(N_TILES):
                    cols = slice(t * C, (t + 1) * C)
                    if t + 2 < T:
                        xp[i][t + 2] = load(i, t + 2)

                    if t < T - 1:
                        # ACT: spike gate z = Relu(BIG*(u* - u))
                        z = pz.tile([128, C], f32, tag="z")
                        nc.scalar.activation(
                            z, u[i], Act.Relu, bias=B_GATE, scale=-BIG
                        )
                        # ACT: o = Relu(1 - z) in {0, 1} -> int8
                        o_t = pout.tile([128, C], i8, tag="o")
                        nc.scalar.activation(
                            o_t, z, Act.Relu, bias=1.0, scale=-1.0
                        )
                        # DVE: w = min(TAU*u, z) == TAU*u*(u <= Vth)
                        w = pw.tile([128, C], f32, tag="w")
                        nc.vector.scalar_tensor_tensor(
                            w, u[i], TAU, z, Alu.mult, Alu.min
                        )
                        # DVE: u' = w + x_{t+1}
                        un = pu.tile([128, C], f32, tag="u")
                        nc.vector.tensor_tensor(un, w, xp[i][t + 1], Alu.add)
                    else:
                        # last step needs no state update: o straight off DVE
                        o_t = pout.tile([128, C], i8, tag="o")
                        nc.vector.tensor_scalar(
                            o_t, u[i], VTH, None, Alu.is_gt
                        )
                    # store from the otherwise-idle GPSIMD HWDGE queue
                    nc.gpsimd.dma_start(y_d[row_sl[i], cols], o_t)
                    if t < T - 1:
                        u[i] = un

    _split_multi_waits(nc)
    return nc


def _shard(x: np.ndarray, c: int) -> np.ndarray:
    """Core c's shard, t-plane-major: [ROWS, C, T] -> [ROWS, T, C] -> flat."""
    s = x[c * B_PER_CORE : (c + 1) * B_PER_CORE].reshape(ROWS, C, T)
    return np.ascontiguousarray(s.transpose(0, 2, 1)).reshape(ROWS, FREE)


def _unshard(y: np.ndarray) -> np.ndarray:
    """Invert _shard's layout for one core's int8 0/1 output -> f32."""
    o = (y > 0).astype(np.float32)
    s = o.reshape(ROWS, T, C).transpose(0, 2, 1)
    return np.ascontiguousarray(s).reshape(B_PER_CORE, *FULL_SHAPE[1:])


def kernel(x: np.ndarray) -> np.ndarray:
    assert x.shape == FULL_SHAPE, x.shape
    in_dtype = x.dtype

    if "nc" not in _cache:
        _cache["nc"] = _build_bass()
    nc = _cache["nc"]

    x = np.ascontiguousarray(x, dtype=np.float32)
    in_maps = [{"x": _shard(x, c)} for c in range(N_CORES)]
    res = run_bass_kernel_spmd(nc, in_maps, core_ids=list(range(N_CORES)))
    out = np.concatenate(
        [_unshard(res.results[c]["y"]) for c in range(N_CORES)], axis=0
    )
    return out.astype(in_dtype, copy=False)
